# revision 2
# baseline (speedup 1.0000x reference)
"""Trainium2 Bass kernel for nn_BackProjection (camera back-projection).

Contract: kernel(**inputs) takes FULL inputs (shp, intrinsics, frustum_masks,
room_masks), returns (kepts (1,256,256,256) bool, mappings (1,256,256,256,5)
float32) — matching reference.reference().

Strategy
--------
The per-voxel transform is affine-separable: depth depends only on z, px on
(x,z), py on (y,z). The host computes small 2D tables (bit-exact replica of
the reference's f32 op chain on CPU jax), and the device does all O(G^3)
work:
- shard x across 8 cores (32 x-columns each)
- fill the mapping output with -1 (ACT engine), overwrite the active z-window
  (~43% of z) with masked channel values via copy_predicated (DVE)
- room-mask lookup: per-partition (y) tables of 2-bit-packed room rows
  indexed by (z, px>>1) via one gpsimd indirect_copy gather per y-half,
  then a 3-op bit extraction on DVE
- stream 44 MB/core back via large contiguous DMAs (memory-bound regime)
"""
import numpy as np

G = 256
NX = 32          # x columns per core
P = 128          # partitions (y half)
XG = 4           # x columns per mapping out-tile
N_CORES = 8
DEPTH_MIN = 0.41
DEPTH_MAX = 6.0
VOXEL_SIZE = 0.05
IMAGE_SIZE = (320, 240)

_cache = {}


def _host_tables(shp, intrinsics):
    """Bit-exact replica of the reference's per-voxel f32 chain on 2D grids."""
    import jax
    import jax.numpy as jnp

    cpu = jax.devices("cpu")[0]
    with jax.default_device(cpu):
        intrinsic = jnp.asarray(intrinsics, jnp.float32)[0]
        intrinsic_inv = jnp.linalg.inv(intrinsic)
        # _generate_frustum
        x, y = float(IMAGE_SIZE[0]), float(IMAGE_SIZE[1])
        corners = jnp.array([[0., 0.], [0., y], [x, y], [x, 0.]], jnp.float32)
        pts = []
        for d in (DEPTH_MIN, DEPTH_MAX):
            p = jnp.concatenate([corners * d,
                                 jnp.full((4, 1), d, jnp.float32),
                                 jnp.ones((4, 1), jnp.float32)], axis=1)
            pts.append(p)
        pts = jnp.concatenate(pts, axis=0)
        frustum = (intrinsic_inv @ pts.T).T[:, :3]
        # _frustum_volume
        mins = jnp.min(frustum, axis=0) / VOXEL_SIZE
        maxs = jnp.max(frustum, axis=0) / VOXEL_SIZE
        dims = jnp.ceil(maxs - mins)
        c2f = (jnp.eye(4, dtype=jnp.float32) / VOXEL_SIZE)
        c2f = c2f.at[3, 3].set(1.0).at[:3, 3].set(-mins)
        padding = (jnp.asarray((G, G, G), jnp.float32) - dims) / 2.0
        f2c = jnp.linalg.inv(c2f)

        ax = jnp.arange(G, dtype=jnp.int32)

        def chain(gidx):
            # gidx: (..., 3) int32 voxel indices; replicates _per_batch ops
            g = gidx.astype(jnp.float32)
            g = jnp.concatenate([float(G) - g[..., :2], g[..., 2:]], axis=-1)
            g = g - padding - 1.0
            gh = jnp.concatenate([g, jnp.ones(g.shape[:-1] + (1,), jnp.float32)],
                                 axis=-1)
            pc = jnp.einsum('ij,xyzj->xyzi', f2c, gh)
            dp = jnp.einsum('ij,xyzj->xyzi', intrinsic, pc)
            depth = dp[..., 2]
            pix = dp[..., :2] / depth[..., None]
            return pix[..., 0], pix[..., 1], depth

        # (x, 1, z) grid with y=0 -> px, depth tables
        gx = jnp.stack(jnp.meshgrid(ax, jnp.arange(1, dtype=jnp.int32), ax,
                                    indexing='ij'), axis=-1)
        px_t, _, depth_t = chain(gx)
        # (1, y, z) grid with x=0 -> py table
        gy = jnp.stack(jnp.meshgrid(jnp.arange(1, dtype=jnp.int32), ax, ax,
                                    indexing='ij'), axis=-1)
        _, py_t, depth_t2 = chain(gy)

    px = np.asarray(px_t)[:, 0, :]          # (x, z) f32
    py = np.asarray(py_t)[0, :, :]          # (y, z) f32
    depth_x = np.asarray(depth_t)[:, 0, :]  # (x, z)
    depth_y = np.asarray(depth_t2)[0]       # (y, z)
    assert np.array_equal(depth_x, np.broadcast_to(depth_x[0:1], depth_x.shape))
    assert np.array_equal(depth_y, np.broadcast_to(depth_y[0:1], depth_y.shape))
    assert np.array_equal(depth_x[0], depth_y[0])
    depth = depth_x[0]                      # (z,)

    shp_f = np.asarray(shp).astype(np.float32)
    w_img, h_img = float(shp_f[1]), float(shp_f[0])

    dok = (depth >= np.float32(DEPTH_MIN)) & (depth <= np.float32(DEPTH_MAX))
    xm = px < w_img                          # (x,z): kept's px condition
    ym = py < h_img                          # (y,z): kept's py condition
    xm2 = xm & (px >= 0.0)                   # kept2's px conditions
    ym2 = ym & (py >= 0.0)                   # kept2's py conditions

    (zidx,) = np.nonzero(dok)
    Z0, ZW = int(zidx[0]), int(len(zidx))
    assert np.array_equal(zidx, np.arange(Z0, Z0 + ZW)), "dok not contiguous"

    pxi = np.where(xm2, px, 0.0).astype(np.int32)   # trunc toward 0; valid in [0,320)
    pyi = np.where(ym2, py, 0.0).astype(np.int32)

    return dict(px=px, py=py, depth=depth, dok=dok, xm=xm, ym=ym, xm2=xm2,
                ym2=ym2, pxi=pxi, pyi=pyi, Z0=Z0, ZW=ZW)


def _build_nc(Z0, ZW, TBLN, reps=1):
    import concourse.bacc as bacc
    import concourse.mybir as mybir
    import concourse.tile as tile
    import concourse.bass as bass

    A = mybir.AluOpType
    AF = mybir.ActivationFunctionType
    dt = mybir.dt
    NW = NX * ZW              # free size of windowed (x,z) tiles
    NIW = NW // 16            # wrapped idx columns

    nc = bacc.Bacc(target_bir_lowering=False)
    # inputs
    fm = nc.dram_tensor("fm", [NX, G, G], dt.uint8, kind="ExternalInput")
    xmkrow = nc.dram_tensor("xmkrow", [NX, ZW], dt.uint8, kind="ExternalInput")
    pxvrow = nc.dram_tensor("pxvrow", [NX, ZW], dt.float32, kind="ExternalInput")
    pxoddrow = nc.dram_tensor("pxoddrow", [NX, ZW], dt.uint8, kind="ExternalInput")
    zrow = nc.dram_tensor("zrow", [1, ZW], dt.float32, kind="ExternalInput")
    drow = nc.dram_tensor("drow", [1, ZW], dt.float32, kind="ExternalInput")
    gidx = nc.dram_tensor("gidx", [P, NIW], dt.uint16, kind="ExternalInput")
    room2 = nc.dram_tensor("room2", [2, P, TBLN], dt.uint8, kind="ExternalInput")
    pyvt = nc.dram_tensor("pyvt", [2, P, G], dt.float32, kind="ExternalInput")
    ymkt = nc.dram_tensor("ymkt", [2, P, G], dt.uint8, kind="ExternalInput")
    ym2t = nc.dram_tensor("ym2t", [2, P, G], dt.uint8, kind="ExternalInput")
    # outputs
    omap = nc.dram_tensor("omap", [NX, G, G, 5], dt.float32, kind="ExternalOutput")
    okept = nc.dram_tensor("okept", [NX, G, G], dt.uint8, kind="ExternalOutput")

    def brow(dram_t, nelem, dtype_len=1):
        # broadcast-read a flat DRAM array of nelem elements to all 128 partitions
        return bass.AP(dram_t, 0, [[0, P], [1, nelem]])

    with tile.TileContext(nc) as tc:
        with tc.tile_pool(name="const", bufs=1) as cpool, \
             tc.tile_pool(name="tabs", bufs=2) as tpool, \
             tc.tile_pool(name="work", bufs=2) as wpool, \
             tc.tile_pool(name="mapp", bufs=3) as mpool:
            # constant (per-core) tiles, loaded once
            t_xmk = cpool.tile([P, NW], dt.uint8, tag="xmk")
            t_pxv = cpool.tile([P, NW], dt.float32, tag="pxv")
            t_pxodd = cpool.tile([P, NW], dt.uint8, tag="pxodd")
            t_z = cpool.tile([P, ZW], dt.float32, tag="zv")
            t_d = cpool.tile([P, ZW], dt.float32, tag="dv")
            t_zero = cpool.tile([P, ZW], dt.float32, tag="zero")
            t_gidx = cpool.tile([P, NIW], dt.uint16, tag="gidx")
            nc.sync.dma_start(t_xmk[:], brow(xmkrow, NW))
            nc.sync.dma_start(t_pxv[:], brow(pxvrow, NW))
            nc.sync.dma_start(t_pxodd[:], brow(pxoddrow, NW))
            nc.sync.dma_start(t_z[:], brow(zrow, ZW))
            nc.sync.dma_start(t_d[:], brow(drow, ZW))
            nc.sync.dma_start(t_gidx[:], gidx[:])
            nc.vector.memset(t_zero[:], 0.0)

            for rep in range(reps):
                for half in range(2):
                    y0 = half * P
                    # per-half tables
                    t_room = tpool.tile([P, TBLN], dt.uint8, tag="room")
                    t_pyv = tpool.tile([P, G], dt.float32, tag="pyv")
                    t_ymk = tpool.tile([P, G], dt.uint8, tag="ymk")
                    t_ym2 = tpool.tile([P, G], dt.uint8, tag="ym2")
                    nc.sync.dma_start(t_room[:], room2[half])
                    nc.sync.dma_start(t_pyv[:], pyvt[half])
                    nc.sync.dma_start(t_ymk[:], ymkt[half])
                    nc.sync.dma_start(t_ym2[:], ym2t[half])

                    # fm window: [128, (x:NX)(z:ZW)] u8
                    t_fm = wpool.tile([P, NW], dt.uint8, tag="fm")
                    src = bass.AP(fm, y0 * G + Z0, [[G, P], [G * G, NX], [1, ZW]])
                    nc.sync.dma_start(t_fm[:], src)

                    def xb(t, width=ZW):
                        # repeat a [P, width] tile along x (free-dim step 0)
                        return bass.AP(t.tensor, t[:].offset,
                                       [t[:].ap[0], [0, NX], [1, width]])

                    # kept mask k = fm & ym & xmk
                    t_k = wpool.tile([P, NW], dt.uint8, tag="k")
                    ymk_win = bass.AP(t_ymk.tensor, t_ymk[:].offset + Z0,
                                      [t_ymk[:].ap[0], [0, NX], [1, ZW]])
                    nc.vector.tensor_tensor(t_k[:], t_fm[:], ymk_win, A.mult)
                    nc.vector.tensor_tensor(t_k[:], t_k[:], t_xmk[:], A.mult)

                    # room gather + 2-bit extraction
                    t_g = wpool.tile([P, NW], dt.uint8, tag="g")
                    CH = 1024
                    for s in range(0, NW, CH):
                        n = min(CH, NW - s)
                        nc.gpsimd.indirect_copy(
                            t_g[:, s:s + n], t_room[:],
                            t_gidx[:, s // 16:(s + n) // 16], True)
                    t_b1 = wpool.tile([P, NW], dt.uint8, tag="b1")
                    nc.vector.tensor_scalar(t_b1[:], t_g[:], 2.0, None, A.is_ge)
                    t_bit = wpool.tile([P, NW], dt.uint8, tag="bit")
                    nc.vector.scalar_tensor_tensor(t_bit[:], t_b1[:], -2.0, t_g[:],
                                                   A.mult, A.add)
                    nc.vector.copy_predicated(t_bit[:], t_pxodd[:], t_b1[:])

                    # kept2 = k & ym2 & bit
                    t_k2 = wpool.tile([P, NW], dt.uint8, tag="k2")
                    ym2_win = bass.AP(t_ym2.tensor, t_ym2[:].offset + Z0,
                                      [t_ym2[:].ap[0], [0, NX], [1, ZW]])
                    nc.vector.tensor_tensor(t_k2[:], t_k[:], ym2_win, A.mult)

                    # kepts out tile
                    t_kept = wpool.tile([P, NX * G], dt.uint8, tag="kept")
                    nc.vector.memset(t_kept[:].bitcast(dt.uint32), 0)
                    kept_win = bass.AP(t_kept.tensor, t_kept[:].offset + Z0,
                                       [t_kept[:].ap[0], [G, NX], [1, ZW]])
                    nc.vector.scalar_tensor_tensor(kept_win, t_k2[:], 1.0,
                                                   t_bit[:], A.mult, A.mult)
                    dstk = bass.AP(okept, y0 * G, [[G, P], [G * G, NX], [1, G]])
                    nc.sync.dma_start(dstk, t_kept[:])

                    # mapping out tiles
                    for xg in range(NX // XG):
                        t_map = mpool.tile([P, XG * G * 5], dt.float32, tag="map")
                        zb = bass.AP(t_zero.tensor, t_zero[:].offset,
                                     [t_zero[:].ap[0], [0, XG * G * 5 // ZW + 1]])
                        zb = bass.AP(t_zero.tensor, t_zero[:].offset,
                                     [t_zero[:].ap[0], [0, XG * G * 5]])
                        nc.scalar.activation(t_map[:], zb, AF.Copy,
                                             bias=-1.0, scale=1.0)
                        x0 = xg * XG
                        k_sl = bass.AP(t_k.tensor, t_k[:].offset + x0 * ZW,
                                       [t_k[:].ap[0], [ZW, XG], [1, ZW]])
                        datas = [
                            bass.AP(t_zero.tensor, t_zero[:].offset,
                                    [t_zero[:].ap[0], [0, XG], [1, ZW]]),       # ch0
                            bass.AP(t_pxv.tensor, t_pxv[:].offset + x0 * ZW,
                                    [t_pxv[:].ap[0], [ZW, XG], [1, ZW]]),       # ch1
                            bass.AP(t_pyv.tensor, t_pyv[:].offset + Z0,
                                    [t_pyv[:].ap[0], [0, XG], [1, ZW]]),        # ch2
                            bass.AP(t_z.tensor, t_z[:].offset,
                                    [t_z[:].ap[0], [0, XG], [1, ZW]]),          # ch3
                            bass.AP(t_d.tensor, t_d[:].offset,
                                    [t_d[:].ap[0], [0, XG], [1, ZW]]),          # ch4
                        ]
                        for c in range(5):
                            out_ap = bass.AP(t_map.tensor,
                                             t_map[:].offset + Z0 * 5 + c,
                                             [t_map[:].ap[0], [G * 5, XG], [5, ZW]])
                            nc.vector.copy_predicated(out_ap, k_sl, datas[c])
                        off = x0 * G * G * 5 + y0 * G * 5
                        dstm = bass.AP(omap, off,
                                       [[G * 5, P], [G * G * 5, XG], [1, G * 5]])
                        nc.sync.dma_start(dstm, t_map[:])
    return nc


class _Runner:
    """Sharded-jit executor over the 8 axon NeuronCores (built once)."""

    def __init__(self, nc, n_cores=N_CORES):
        import jax
        import numpy as _np
        from jax.sharding import Mesh, PartitionSpec
        from jax.experimental.shard_map import shard_map
        import concourse.mybir as mybir
        from concourse.bass2jax import (_bass_exec_p, partition_id_tensor,
                                        install_neuronx_cc_hook)
        install_neuronx_cc_hook()
        if not nc.is_finalized():
            nc.compile()
            nc.finalize()
        self.n_cores = n_cores
        pname = nc.partition_id_tensor.name if nc.partition_id_tensor else None
        in_names, out_names, out_avals = [], [], []
        for alloc in nc.m.functions[0].allocations:
            if not isinstance(alloc, mybir.MemoryLocationSet):
                continue
            name = alloc.memorylocations[0].name
            if alloc.kind == "ExternalInput":
                if name != pname:
                    in_names.append(name)
            elif alloc.kind == "ExternalOutput":
                out_names.append(name)
                out_avals.append(jax.core.ShapedArray(
                    tuple(alloc.tensor_shape), mybir.dt.np(alloc.dtype)))
        self.in_names, self.out_names, self.out_avals = in_names, out_names, out_avals
        bind_names = in_names + out_names + ([pname] if pname else [])

        def _body(*args):
            operands = list(args)
            if pname is not None:
                operands.append(partition_id_tensor())
            outs = _bass_exec_p.bind(
                *operands, out_avals=tuple(out_avals),
                in_names=tuple(bind_names), out_names=tuple(out_names),
                lowering_input_output_aliases=(),
                sim_require_finite=True, sim_require_nnan=True, nc=nc)
            return tuple(outs)

        devices = jax.devices()[:n_cores]
        mesh = Mesh(_np.asarray(devices), ("core",))
        n_args = len(in_names) + len(out_names)
        self.fn = jax.jit(
            shard_map(_body, mesh=mesh,
                      in_specs=(PartitionSpec("core"),) * n_args,
                      out_specs=(PartitionSpec("core"),) * len(out_names),
                      check_rep=False),
            keep_unused=True)
        self.jax = jax

    def run(self, in_maps):
        np_ = np
        concat = [np_.concatenate([np_.asarray(m[n]) for m in in_maps], axis=0)
                  for n in self.in_names]
        for aval in self.out_avals:
            concat.append(np_.zeros((self.n_cores * aval.shape[0], *aval.shape[1:]),
                                    aval.dtype))
        outs = self.fn(*concat)
        self.jax.block_until_ready(outs)
        res = []
        for c in range(self.n_cores):
            d = {}
            for i, n in enumerate(self.out_names):
                arr = np_.asarray(outs[i])
                d[n] = arr.reshape(self.n_cores, *self.out_avals[i].shape)[c]
            res.append(d)
        return res


def _make_in_maps(tabs, frustum_masks, room_masks):
    Z0, ZW = tabs["Z0"], tabs["ZW"]
    TBLN = ZW * 160 + 16
    DEADIDX = ZW * 160

    fm_full = np.asarray(frustum_masks)[0].astype(np.uint8)      # (256,256,256)
    room = np.asarray(room_masks)[0, 0].astype(np.uint8)         # (240,320)

    # 2-bit packed room rows: packed2[r, c] = room[r,2c] + 2*room[r,2c+1]
    packed2 = (room[:, 0::2] + 2 * room[:, 1::2]).astype(np.uint8)  # (240,160)

    # per-(y,z) table rows
    pyi = tabs["pyi"]                    # (y,z) int32 (0 where invalid)
    room2 = np.zeros((2, P, TBLN), np.uint8)
    rows = packed2[pyi[:, Z0:Z0 + ZW]]   # (256, ZW, 160)
    rows = rows * tabs["ym2"][:, Z0:Z0 + ZW, None].astype(np.uint8)
    room2[0, :, :ZW * 160] = rows[:P].reshape(P, ZW * 160)
    room2[1, :, :ZW * 160] = rows[P:].reshape(P, ZW * 160)

    pyv = tabs["py"].astype(np.float32)
    pyvt = np.stack([pyv[:P], pyv[P:]]).copy()                   # (2,128,256)
    ymkt = np.stack([tabs["ym"][:P], tabs["ym"][P:]]).astype(np.uint8)
    ym2t = np.stack([tabs["ym2"][:P], tabs["ym2"][P:]]).astype(np.uint8)

    zrow = np.arange(Z0, Z0 + ZW, dtype=np.float32)[None, :]
    drow = tabs["depth"][None, Z0:Z0 + ZW].astype(np.float32)

    xmk = (tabs["xm"] & tabs["dok"][None, :])                    # (x,z)
    xm2k = (tabs["xm2"] & tabs["dok"][None, :])
    pxi = tabs["pxi"]

    in_maps = []
    for c in range(N_CORES):
        xs = slice(NX * c, NX * (c + 1))
        idx_flat = np.where(xm2k[xs, Z0:Z0 + ZW],
                            (np.arange(ZW)[None, :] * 160 + (pxi[xs, Z0:Z0 + ZW] >> 1)),
                            DEADIDX).astype(np.uint16).reshape(-1)   # (NX*ZW,)
        n = idx_flat.shape[0]
        gidx = np.zeros((P, n // 16), np.uint16)
        ii = np.arange(n)
        for grp in range(8):
            gidx[16 * grp + ii % 16, ii // 16] = idx_flat
        in_maps.append(dict(
            fm=fm_full[xs].copy(),
            xmkrow=xmk[xs, Z0:Z0 + ZW].astype(np.uint8).copy(),
            pxvrow=tabs["px"][xs, Z0:Z0 + ZW].astype(np.float32).copy(),
            pxoddrow=(pxi[xs, Z0:Z0 + ZW] & 1).astype(np.uint8).copy(),
            zrow=zrow, drow=drow, gidx=gidx,
            room2=room2, pyvt=pyvt, ymkt=ymkt, ym2t=ym2t,
        ))
    return in_maps, TBLN


def kernel(shp, intrinsics, frustum_masks, room_masks):
    shp = np.asarray(shp)
    intrinsics = np.asarray(intrinsics)
    assert intrinsics.shape[0] == 1, "kernel hardcodes B=1"

    tabs = _host_tables(shp, intrinsics)
    in_maps, TBLN = _make_in_maps(tabs, frustum_masks, room_masks)

    key = (tabs["Z0"], tabs["ZW"], TBLN)
    if key not in _cache:
        nc = _build_nc(tabs["Z0"], tabs["ZW"], TBLN)
        _cache[key] = _Runner(nc)
    res = _cache[key].run(in_maps)

    kepts = np.concatenate([r["okept"] for r in res], axis=0)[None]
    mappings = np.concatenate([r["omap"] for r in res], axis=0)[None]
    return kepts.astype(bool), mappings
